# revision 16
# baseline (speedup 1.0000x reference)
"""Conv2d 3x3 VALID kernel for Trainium2, batch-sharded across 8 NeuronCores.

Problem: input [32,128,64,64] f32, weights [256,128,3,3] f32 ->
output [32,256,62,62] f32 (stride 1, no padding).

Strategy (per core, 4 images):
  - Cin=128 == SBUF partition dim == matmul contraction dim.
  - Input image b is DMA'd as fp32 [128, 4096] (row-major h*64+w) and cast
    to bf16 on the otherwise-idle scalar engine.
  - out[y, x] = sum_{kh,kw,ci} in[ci, (y+kh)*64 + x+kw] * W[co,ci,kh,kw].
    For a block of 8 output rows and tap (kh,kw), the rhs is the strided AP
    in_bf[:, (y0+kh)*64+kw :][8 rows step 64, 62 cols step 1] -> N=496
    moving columns, accumulated over the 9 taps into one PSUM bank
    (fp32 accumulation).
  - Cout=256 -> two halves of 128 (PSUM partition limit).
  - Weights are DMA'd raw [co,(ci kh kw)], transposed on-chip with PE
    transposes, and cast to bf16 in the PSUM->SBUF copy, giving lhsT
    layout [ci, tap*256 + half*128 + co].

Why bf16 (trace-driven): with fp32r operands the LDWEIGHTS (187ns, no
fast-weight-load for 4-byte dtypes) bounds the matmul cadence at ~233ns
vs the 207ns stream floor for N=496. bf16 weights enable FWL (~2x faster
load) so the cadence drops to the stream rate; bf16 also keeps 1 cyc/row
streaming. Max-rel-error vs the fp32 reference is ~2.2e-3 (measured on
the real data), well under the 2e-2 gate.

Other perf notes:
  - Tile's per-MM tick increments are elided inside 9-tap accumulation
    groups (_elide_mm_ticks) with all wait thresholds renumbered: only
    group-final counts are ever waited on.
  - Head: weights DMA split per Cout-half; image 0 DMA'd in staircase
    pieces so each row-block's rows land just before the conv stream
    reaches it; h=1 transposes deferred past the h=0 conv; a few fp32
    warm-up matmuls on the identity keep the PE busy through the DMA head
    so the HAM clock gate (1.2->2.4GHz after ~3.4us busy) is released
    when the conv stream starts.
  - Tap-outer/multi-bank interleaving was tried and is ~43ns/MM SLOWER
    (PSUM bank cycling penalty); keep the 9 taps of one block consecutive.
"""

import os as _os

import numpy as np

import bass_rust
import concourse.bass as bass
import concourse.mybir as mybir
import concourse.tile as tile
from concourse import bacc
from concourse.bass_utils import run_bass_kernel_spmd
from concourse.masks import make_identity

F32 = mybir.dt.float32
BF16 = mybir.dt.bfloat16

B, CIN, H, W = 32, 128, 64, 64
COUT, KH, KW = 256, 3, 3
OH, OW = H - KH + 1, W - KW + 1  # 62, 62
N_CORES = 8
BL = B // N_CORES  # 4 images per core

IMG_STRIDE = H * W  # 4096
W_FREE = CIN * KH * KW  # 1152
N_TAPS = KH * KW  # 9
ROWS_PER_CHUNK = 8  # 8 output rows x 62 cols = 496 <= 512 (one PSUM bank)
N_WARMUP = 5  # fp32 warm-up matmuls on the identity

ELIDE = _os.environ.get("K_ELIDE", "1") == "1"
# staircase pieces for image 0 (fp32 column ranges); piece k covers the
# input rows needed by row-block k of the first image-half
IMG0_PIECES = [(0, 640), (640, 1280), (1280, 2048), (2048, 4096)]


def _conv_body(nc, tc, out_d, x_d, w_d):
    x_r = x_d.rearrange("b c h w -> b c (h w)")  # [BL, 128, 4096]
    w_r = w_d.rearrange("co ci kh kw -> co (ci kh kw)")  # [256, 1152]

    with (
        tc.tile_pool(name="const", bufs=1) as cpool,
        tc.tile_pool(name="psum", bufs=8, space=bass.MemorySpace.PSUM) as psum_pool,
        tc.tile_pool(name="outp", bufs=6) as out_pool,
    ):
        in_sb = cpool.tile([128, BL * IMG_STRIDE], F32)
        in_bf = cpool.tile([128, BL * IMG_STRIDE], BF16)
        w_raw = cpool.tile([128, 2 * W_FREE], F32)
        w_l = cpool.tile([128, N_TAPS * COUT], BF16)  # [ci, t*256 + h*128 + co]
        ident = cpool.tile([128, 128], F32)

        make_identity(nc, ident)

        # Warm-up: plain fp32 matmuls (4 cyc/row) on the identity keep the
        # PE busy through the weights-DMA wait so the HAM clock gate is
        # released before the conv stream starts.
        for _ in range(N_WARMUP):
            wps = psum_pool.tile([128, 512], F32, tag="ps")
            nc.tensor.matmul(wps[:, :128], ident, ident, start=True, stop=True)

        # TRN2 has two physical HW-DGE rings (SP and Activation). Weights
        # ride the Activation ring so they land concurrently with the
        # image pieces on the SP ring instead of serializing behind them.
        nc.scalar.dma_start(
            out=w_raw[:, :W_FREE],
            in_=w_r.rearrange("(h p) c -> h p c", h=2)[0],
        )
        nc.scalar.dma_start(
            out=w_raw[:, W_FREE:],
            in_=w_r.rearrange("(h p) c -> h p c", h=2)[1],
        )
        for c0, c1 in IMG0_PIECES:
            nc.sync.dma_start(out=in_sb[:, c0:c1], in_=x_r[0][:, c0:c1])
        for b in range(1, BL):
            nc.sync.dma_start(
                out=in_sb[:, b * IMG_STRIDE : (b + 1) * IMG_STRIDE],
                in_=x_r[b],
            )

        # fp32 -> bf16 input casts on the (otherwise idle) scalar engine,
        # piece-wise for image 0 so the conv stream can start early.
        for c0, c1 in IMG0_PIECES:
            nc.scalar.copy(in_bf[:, c0:c1], in_sb[:, c0:c1])
        for b in range(1, BL):
            nc.scalar.copy(
                in_bf[:, b * IMG_STRIDE : (b + 1) * IMG_STRIDE],
                in_sb[:, b * IMG_STRIDE : (b + 1) * IMG_STRIDE],
            )

        # Transpose weights: w_raw half h viewed as [co, (ci t)] -> per tap
        # [co, ci] (ci at stride 9) -> PE transpose -> bf16 [ci, co].
        def transpose_half(h):
            w_v = w_raw[:, h * W_FREE : (h + 1) * W_FREE].rearrange(
                "p (ci t) -> p t ci", t=N_TAPS
            )
            for t in range(N_TAPS):
                ps = psum_pool.tile([128, 512], F32, tag="ps")
                nc.tensor.transpose(ps[:, :128], w_v[:, t, :], ident)
                nc.vector.tensor_copy(
                    w_l[:, t * COUT + h * 128 : t * COUT + h * 128 + 128],
                    ps[:, :128],
                )

        def conv_half(h):
            for b in range(BL):
                img_v = in_bf[
                    :, b * IMG_STRIDE : (b + 1) * IMG_STRIDE
                ].rearrange("p (r x) -> p r x", x=W)  # [128, 64, 64]
                for y0 in range(0, OH, ROWS_PER_CHUNK):
                    nrows = min(ROWS_PER_CHUNK, OH - y0)
                    size = nrows * OW
                    ps = psum_pool.tile([128, 512], F32, tag="ps")
                    ps_v = ps[:, :size].rearrange("p (r x) -> p r x", x=OW)
                    for t in range(N_TAPS):
                        kh, kw = divmod(t, KW)
                        lhsT = w_l[:, t * COUT + h * 128 : t * COUT + h * 128 + 128]
                        rhs = img_v[:, y0 + kh : y0 + kh + nrows, kw : kw + OW]
                        nc.tensor.matmul(
                            ps_v,
                            lhsT,
                            rhs,
                            start=(t == 0),
                            stop=(t == N_TAPS - 1),
                        )
                    ot = out_pool.tile([128, ROWS_PER_CHUNK * OW], F32)
                    nc.vector.tensor_copy(ot[:, :size], ps[:, :size])
                    nc.sync.dma_start(
                        out=out_d[b, h * 128 : (h + 1) * 128, y0 : y0 + nrows, :],
                        in_=ot[:, :size].rearrange("p (r x) -> p r x", x=OW),
                    )

        transpose_half(0)
        conv_half(0)
        transpose_half(1)
        conv_half(1)


def _elide_mm_ticks(nc):
    """Remove the per-MM tile-tick semaphore increments inside 9-tap matmul
    accumulation groups and renumber every wait threshold on that sem.

    Tile gives every instruction a completion increment on its engine's
    tick semaphore, but consumers only ever wait on the cumulative count
    reached at a group-final MM. Walrus requires UpdateValue == 1, so we
    drop the intermediate increments and rewrite all waits into the new
    (kept increments only) numbering. A wait that referenced an elided
    count is bumped to the next kept increment (the group-final MM), which
    completes at-or-after the original trigger point.
    """
    insts_all = [
        i for fn in nc.m.functions for blk in fn.blocks for i in blk.instructions
    ]

    pe_sem_ids = set()
    for i in insts_all:
        if isinstance(i, mybir.InstMatmult) and i.sync_info is not None:
            for u in i.sync_info.on_update:
                if u.update_mode == "sem-inc":
                    pe_sem_ids.add(u.id)
    assert len(pe_sem_ids) == 1, pe_sem_ids
    pe_sem = next(iter(pe_sem_ids))

    def pe_incs(i):
        if i.sync_info is None:
            return []
        return [
            u
            for u in i.sync_info.on_update
            if u.id == pe_sem and u.update_mode == "sem-inc"
        ]

    incs = [i for i in insts_all if pe_incs(i)]
    for i in incs:
        ups = pe_incs(i)
        assert len(ups) == 1 and ups[0].update_value == 1, (i.name, ups)

    waited = set()
    for i in insts_all:
        si = i.sync_info
        if si is None:
            continue
        for w in si.on_wait:
            if w.id == pe_sem:
                assert w.wait_mode == "sem-ge-imm" and w.wait_reg is None, w
                waited.add(w.wait_value)

    kept = []
    for old_cum, i in enumerate(incs, start=1):
        elide = (
            isinstance(i, mybir.InstMatmult)
            and not i.is_transpose
            and not i.stop_tensor_calc
            and old_cum not in waited
        )
        kept.append(not elide)
    newc = []
    c = 0
    for k in kept:
        c += k
        newc.append(c)

    n_elided = 0
    for i, k in zip(incs, kept):
        if not k:
            si = i.sync_info
            i.sync_info = bass_rust.SyncInfo(
                on_wait=list(si.on_wait),
                on_update=[
                    u
                    for u in si.on_update
                    if not (u.id == pe_sem and u.update_mode == "sem-inc")
                ],
            )
            n_elided += 1

    for i in insts_all:
        si = i.sync_info
        if si is None:
            continue
        for w in si.on_wait:
            if w.id == pe_sem:
                v = w.wait_value
                assert 1 <= v <= len(incs), (i.name, v)
                w.wait_value = newc[v - 1] + (0 if kept[v - 1] else 1)
    return n_elided


def build_module(elide_ticks=ELIDE):
    nc = bacc.Bacc(
        "TRN2", target_bir_lowering=False, debug=False, num_devices=N_CORES
    )
    x_d = nc.dram_tensor(
        "input_image", [BL, CIN, H, W], F32, kind="ExternalInput"
    ).ap()
    w_d = nc.dram_tensor("weights", [COUT, CIN, KH, KW], F32, kind="ExternalInput").ap()
    out_d = nc.dram_tensor("out", [BL, COUT, OH, OW], F32, kind="ExternalOutput").ap()
    with tile.TileContext(nc) as tc:
        _conv_body(nc, tc, out_d, x_d, w_d)
    if elide_ticks:
        try:
            _elide_mm_ticks(nc)
        except Exception:
            # the pass mutates sync_info in place; on any unexpected BIR
            # shape discard this module and rebuild without the elision
            return build_module(elide_ticks=False)
    nc.compile()
    return nc


_NC_CACHE = {}


def _get_module():
    if "nc" not in _NC_CACHE:
        _NC_CACHE["nc"] = build_module()
    return _NC_CACHE["nc"]


def kernel(input_image: np.ndarray, weights: np.ndarray) -> np.ndarray:
    input_image = np.ascontiguousarray(input_image, dtype=np.float32)
    weights = np.ascontiguousarray(weights, dtype=np.float32)
    nc = _get_module()
    in_maps = [
        {
            "input_image": input_image[i * BL : (i + 1) * BL],
            "weights": weights,
        }
        for i in range(N_CORES)
    ]
    res = run_bass_kernel_spmd(nc, in_maps, list(range(N_CORES))).results
    return np.concatenate([r["out"] for r in res], axis=0)


# revision 17
# speedup vs baseline: 1.1887x; 1.1887x over previous
"""Conv2d 3x3 VALID kernel for Trainium2, batch-sharded across 8 NeuronCores.

Problem: input [32,128,64,64] f32, weights [256,128,3,3] f32 ->
output [32,256,62,62] f32 (stride 1, no padding).

Strategy (per core, 4 images):
  - Cin=128 == SBUF partition dim == matmul contraction dim.
  - Input image b is DMA'd as fp32 [128, 4096] (row-major h*64+w) and cast
    to bf16 on the otherwise-idle scalar engine.
  - out[y, x] = sum_{kh,kw,ci} in[ci, (y+kh)*64 + x+kw] * W[co,ci,kh,kw].
    For a block of 8 output rows and tap (kh,kw), the rhs is the strided AP
    in_bf[:, (y0+kh)*64+kw :][8 rows step 64, 62 cols step 1] -> N=496
    moving columns, accumulated over the 9 taps into one PSUM bank
    (fp32 accumulation).
  - Cout=256 -> two halves of 128 (PSUM partition limit).
  - Weights are DMA'd raw [co,(ci kh kw)], transposed on-chip with PE
    transposes, and cast to bf16 in the PSUM->SBUF copy, giving lhsT
    layout [ci, tap*256 + half*128 + co].

Why bf16 (trace-driven): with fp32r operands the LDWEIGHTS (187ns, no
fast-weight-load for 4-byte dtypes) bounds the matmul cadence at ~233ns
vs the 207ns stream floor for N=496. bf16 weights enable FWL (~2x faster
load) so the cadence drops to the stream rate; bf16 also keeps 1 cyc/row
streaming. Max-rel-error vs the fp32 reference is ~2.2e-3 (measured on
the real data), well under the 2e-2 gate.

Other perf notes:
  - Tile's per-MM tick increments are elided inside 9-tap accumulation
    groups (_elide_mm_ticks) with all wait thresholds renumbered: only
    group-final counts are ever waited on.
  - Head: weights DMA split per Cout-half; image 0 DMA'd in staircase
    pieces so each row-block's rows land just before the conv stream
    reaches it; h=1 transposes deferred past the h=0 conv; a few fp32
    warm-up matmuls on the identity keep the PE busy through the DMA head
    so the HAM clock gate (1.2->2.4GHz after ~3.4us busy) is released
    when the conv stream starts.
  - Tap-outer/multi-bank interleaving was tried and is ~43ns/MM SLOWER
    (PSUM bank cycling penalty); keep the 9 taps of one block consecutive.
"""

import os as _os

import numpy as np

import bass_rust
import concourse.bass as bass
import concourse.mybir as mybir
import concourse.tile as tile
from concourse import bacc
from concourse.bass_utils import run_bass_kernel_spmd
from concourse.masks import make_identity

F32 = mybir.dt.float32
BF16 = mybir.dt.bfloat16

B, CIN, H, W = 32, 128, 64, 64
COUT, KH, KW = 256, 3, 3
OH, OW = H - KH + 1, W - KW + 1  # 62, 62
N_CORES = 8
BL = B // N_CORES  # 4 images per core

IMG_STRIDE = H * W  # 4096
W_FREE = CIN * KH * KW  # 1152
N_TAPS = KH * KW  # 9
ROWS_PER_CHUNK = 8  # 8 output rows x 62 cols = 496 <= 512 (one PSUM bank)
N_WARMUP = 5  # fp32 warm-up matmuls on the identity

ELIDE = _os.environ.get("K_ELIDE", "1") == "1"
# staircase pieces for image 0 (fp32 column ranges); piece k covers the
# input rows needed by row-block k of the first image-half
IMG0_PIECES = [(0, 640), (640, 1280), (1280, 2048), (2048, 4096)]


def _conv_body(nc, tc, out_d, x_d, w_d):
    x_r = x_d.rearrange("b c h w -> b c (h w)")  # [BL, 128, 4096]
    w_r = w_d.rearrange("co ci kh kw -> co (ci kh kw)")  # [256, 1152]

    with (
        tc.tile_pool(name="const", bufs=1) as cpool,
        tc.tile_pool(name="psum", bufs=8, space=bass.MemorySpace.PSUM) as psum_pool,
        tc.tile_pool(name="outp", bufs=6) as out_pool,
    ):
        in_sb = cpool.tile([128, BL * IMG_STRIDE], F32)
        in_bf = cpool.tile([128, BL * IMG_STRIDE], BF16)
        w_raw = cpool.tile([128, 2 * W_FREE], F32)
        w_l = cpool.tile([128, N_TAPS * COUT], BF16)  # [ci, t*256 + h*128 + co]
        ident = cpool.tile([128, 128], F32)

        make_identity(nc, ident)

        # Warm-up: plain fp32 matmuls (4 cyc/row) on the identity keep the
        # PE busy through the weights-DMA wait so the HAM clock gate is
        # released before the conv stream starts.
        for _ in range(N_WARMUP):
            wps = psum_pool.tile([128, 512], F32, tag="ps")
            nc.tensor.matmul(wps[:, :128], ident, ident, start=True, stop=True)

        # DMA issue order = transfer completion order (queues drain FIFO):
        # h0 weights, image-0 staircase, h1 weights, remaining images.
        # (Issuing the weight DMAs on the Activation HW-DGE ring instead
        # was tried and regresses badly: conv cadence 209->251ns.)
        nc.sync.dma_start(
            out=w_raw[:, :W_FREE],
            in_=w_r.rearrange("(h p) c -> h p c", h=2)[0],
        )
        for c0, c1 in IMG0_PIECES:
            nc.sync.dma_start(out=in_sb[:, c0:c1], in_=x_r[0][:, c0:c1])
        nc.sync.dma_start(
            out=w_raw[:, W_FREE:],
            in_=w_r.rearrange("(h p) c -> h p c", h=2)[1],
        )
        for b in range(1, BL):
            nc.sync.dma_start(
                out=in_sb[:, b * IMG_STRIDE : (b + 1) * IMG_STRIDE],
                in_=x_r[b],
            )

        # fp32 -> bf16 input casts on the (otherwise idle) scalar engine,
        # piece-wise for image 0 so the conv stream can start early.
        for c0, c1 in IMG0_PIECES:
            nc.scalar.copy(in_bf[:, c0:c1], in_sb[:, c0:c1])
        for b in range(1, BL):
            nc.scalar.copy(
                in_bf[:, b * IMG_STRIDE : (b + 1) * IMG_STRIDE],
                in_sb[:, b * IMG_STRIDE : (b + 1) * IMG_STRIDE],
            )

        # Transpose weights: w_raw half h viewed as [co, (ci t)] -> per tap
        # [co, ci] (ci at stride 9) -> PE transpose -> bf16 [ci, co].
        def transpose_half(h):
            w_v = w_raw[:, h * W_FREE : (h + 1) * W_FREE].rearrange(
                "p (ci t) -> p t ci", t=N_TAPS
            )
            for t in range(N_TAPS):
                ps = psum_pool.tile([128, 512], F32, tag="ps")
                nc.tensor.transpose(ps[:, :128], w_v[:, t, :], ident)
                nc.vector.tensor_copy(
                    w_l[:, t * COUT + h * 128 : t * COUT + h * 128 + 128],
                    ps[:, :128],
                )

        def conv_half(h):
            for b in range(BL):
                img_v = in_bf[
                    :, b * IMG_STRIDE : (b + 1) * IMG_STRIDE
                ].rearrange("p (r x) -> p r x", x=W)  # [128, 64, 64]
                for y0 in range(0, OH, ROWS_PER_CHUNK):
                    nrows = min(ROWS_PER_CHUNK, OH - y0)
                    size = nrows * OW
                    ps = psum_pool.tile([128, 512], F32, tag="ps")
                    ps_v = ps[:, :size].rearrange("p (r x) -> p r x", x=OW)
                    for t in range(N_TAPS):
                        kh, kw = divmod(t, KW)
                        lhsT = w_l[:, t * COUT + h * 128 : t * COUT + h * 128 + 128]
                        rhs = img_v[:, y0 + kh : y0 + kh + nrows, kw : kw + OW]
                        nc.tensor.matmul(
                            ps_v,
                            lhsT,
                            rhs,
                            start=(t == 0),
                            stop=(t == N_TAPS - 1),
                        )
                    ot = out_pool.tile([128, ROWS_PER_CHUNK * OW], F32)
                    nc.vector.tensor_copy(ot[:, :size], ps[:, :size])
                    nc.sync.dma_start(
                        out=out_d[b, h * 128 : (h + 1) * 128, y0 : y0 + nrows, :],
                        in_=ot[:, :size].rearrange("p (r x) -> p r x", x=OW),
                    )

        transpose_half(0)
        conv_half(0)
        transpose_half(1)
        conv_half(1)


def _elide_mm_ticks(nc):
    """Remove the per-MM tile-tick semaphore increments inside 9-tap matmul
    accumulation groups and renumber every wait threshold on that sem.

    Tile gives every instruction a completion increment on its engine's
    tick semaphore, but consumers only ever wait on the cumulative count
    reached at a group-final MM. Walrus requires UpdateValue == 1, so we
    drop the intermediate increments and rewrite all waits into the new
    (kept increments only) numbering. A wait that referenced an elided
    count is bumped to the next kept increment (the group-final MM), which
    completes at-or-after the original trigger point.
    """
    insts_all = [
        i for fn in nc.m.functions for blk in fn.blocks for i in blk.instructions
    ]

    pe_sem_ids = set()
    for i in insts_all:
        if isinstance(i, mybir.InstMatmult) and i.sync_info is not None:
            for u in i.sync_info.on_update:
                if u.update_mode == "sem-inc":
                    pe_sem_ids.add(u.id)
    assert len(pe_sem_ids) == 1, pe_sem_ids
    pe_sem = next(iter(pe_sem_ids))

    def pe_incs(i):
        if i.sync_info is None:
            return []
        return [
            u
            for u in i.sync_info.on_update
            if u.id == pe_sem and u.update_mode == "sem-inc"
        ]

    incs = [i for i in insts_all if pe_incs(i)]
    for i in incs:
        ups = pe_incs(i)
        assert len(ups) == 1 and ups[0].update_value == 1, (i.name, ups)

    waited = set()
    for i in insts_all:
        si = i.sync_info
        if si is None:
            continue
        for w in si.on_wait:
            if w.id == pe_sem:
                assert w.wait_mode == "sem-ge-imm" and w.wait_reg is None, w
                waited.add(w.wait_value)

    kept = []
    for old_cum, i in enumerate(incs, start=1):
        elide = (
            isinstance(i, mybir.InstMatmult)
            and not i.is_transpose
            and not i.stop_tensor_calc
            and old_cum not in waited
        )
        kept.append(not elide)
    newc = []
    c = 0
    for k in kept:
        c += k
        newc.append(c)

    n_elided = 0
    for i, k in zip(incs, kept):
        if not k:
            si = i.sync_info
            i.sync_info = bass_rust.SyncInfo(
                on_wait=list(si.on_wait),
                on_update=[
                    u
                    for u in si.on_update
                    if not (u.id == pe_sem and u.update_mode == "sem-inc")
                ],
            )
            n_elided += 1

    for i in insts_all:
        si = i.sync_info
        if si is None:
            continue
        for w in si.on_wait:
            if w.id == pe_sem:
                v = w.wait_value
                assert 1 <= v <= len(incs), (i.name, v)
                w.wait_value = newc[v - 1] + (0 if kept[v - 1] else 1)
    return n_elided


def build_module(elide_ticks=ELIDE):
    nc = bacc.Bacc(
        "TRN2", target_bir_lowering=False, debug=False, num_devices=N_CORES
    )
    x_d = nc.dram_tensor(
        "input_image", [BL, CIN, H, W], F32, kind="ExternalInput"
    ).ap()
    w_d = nc.dram_tensor("weights", [COUT, CIN, KH, KW], F32, kind="ExternalInput").ap()
    out_d = nc.dram_tensor("out", [BL, COUT, OH, OW], F32, kind="ExternalOutput").ap()
    with tile.TileContext(nc) as tc:
        _conv_body(nc, tc, out_d, x_d, w_d)
    if elide_ticks:
        try:
            _elide_mm_ticks(nc)
        except Exception:
            # the pass mutates sync_info in place; on any unexpected BIR
            # shape discard this module and rebuild without the elision
            return build_module(elide_ticks=False)
    nc.compile()
    return nc


_NC_CACHE = {}


def _get_module():
    if "nc" not in _NC_CACHE:
        _NC_CACHE["nc"] = build_module()
    return _NC_CACHE["nc"]


def kernel(input_image: np.ndarray, weights: np.ndarray) -> np.ndarray:
    input_image = np.ascontiguousarray(input_image, dtype=np.float32)
    weights = np.ascontiguousarray(weights, dtype=np.float32)
    nc = _get_module()
    in_maps = [
        {
            "input_image": input_image[i * BL : (i + 1) * BL],
            "weights": weights,
        }
        for i in range(N_CORES)
    ]
    res = run_bass_kernel_spmd(nc, in_maps, list(range(N_CORES))).results
    return np.concatenate([r["out"] for r in res], axis=0)


# revision 18
# speedup vs baseline: 1.2002x; 1.0097x over previous
"""Conv2d 3x3 VALID kernel for Trainium2, batch-sharded across 8 NeuronCores.

Problem: input [32,128,64,64] f32, weights [256,128,3,3] f32 ->
output [32,256,62,62] f32 (stride 1, no padding).

Strategy (per core, 4 images):
  - Cin=128 == SBUF partition dim == matmul contraction dim.
  - Input image b is DMA'd as fp32 [128, 4096] (row-major h*64+w) and cast
    to bf16 on the otherwise-idle scalar engine.
  - out[y, x] = sum_{kh,kw,ci} in[ci, (y+kh)*64 + x+kw] * W[co,ci,kh,kw].
    For a block of 8 output rows and tap (kh,kw), the rhs is the strided AP
    in_bf[:, (y0+kh)*64+kw :][8 rows step 64, 62 cols step 1] -> N=496
    moving columns, accumulated over the 9 taps into one PSUM bank
    (fp32 accumulation).
  - Cout=256 -> two halves of 128 (PSUM partition limit).
  - Weights are DMA'd raw [co,(ci kh kw)], transposed on-chip with PE
    transposes, and cast to bf16 in the PSUM->SBUF copy, giving lhsT
    layout [ci, tap*256 + half*128 + co].

Why bf16 (trace-driven): with fp32r operands the LDWEIGHTS (187ns, no
fast-weight-load for 4-byte dtypes) bounds the matmul cadence at ~233ns
vs the 207ns stream floor for N=496. bf16 weights enable FWL (~2x faster
load) so the cadence drops to the stream rate; bf16 also keeps 1 cyc/row
streaming. Max-rel-error vs the fp32 reference is ~2.2e-3 (measured on
the real data), well under the 2e-2 gate.

Other perf notes:
  - Tile's per-MM tick increments are elided inside 9-tap accumulation
    groups (_elide_mm_ticks) with all wait thresholds renumbered: only
    group-final counts are ever waited on.
  - Head: weights DMA split per Cout-half; image 0 DMA'd in staircase
    pieces so each row-block's rows land just before the conv stream
    reaches it; h=1 transposes deferred past the h=0 conv; a few fp32
    warm-up matmuls on the identity keep the PE busy through the DMA head
    so the HAM clock gate (1.2->2.4GHz after ~3.4us busy) is released
    when the conv stream starts.
  - Tap-outer/multi-bank interleaving was tried and is ~43ns/MM SLOWER
    (PSUM bank cycling penalty); keep the 9 taps of one block consecutive.
"""

import os as _os

import numpy as np

import bass_rust
import concourse.bass as bass
import concourse.mybir as mybir
import concourse.tile as tile
from concourse import bacc
from concourse.bass_utils import run_bass_kernel_spmd
from concourse.masks import make_identity

F32 = mybir.dt.float32
BF16 = mybir.dt.bfloat16

B, CIN, H, W = 32, 128, 64, 64
COUT, KH, KW = 256, 3, 3
OH, OW = H - KH + 1, W - KW + 1  # 62, 62
N_CORES = 8
BL = B // N_CORES  # 4 images per core

IMG_STRIDE = H * W  # 4096
W_FREE = CIN * KH * KW  # 1152
N_TAPS = KH * KW  # 9
ROWS_PER_CHUNK = 8  # 8 output rows x 62 cols = 496 <= 512 (one PSUM bank)
N_WARMUP = 5  # fp32 warm-up matmuls on the identity

ELIDE = _os.environ.get("K_ELIDE", "1") == "1"
# staircase pieces for image 0 (fp32 column ranges); piece k covers the
# input rows needed by row-block k of the first image-half
IMG0_PIECES = [(0, 640), (640, 1280), (1280, 2048), (2048, 4096)]


def _conv_body(nc, tc, out_d, x_d, w_d):
    x_r = x_d.rearrange("b c h w -> b c (h w)")  # [BL, 128, 4096]
    w_r = w_d.rearrange("co ci kh kw -> co (ci kh kw)")  # [256, 1152]

    with (
        tc.tile_pool(name="const", bufs=1) as cpool,
        tc.tile_pool(name="psum", bufs=8, space=bass.MemorySpace.PSUM) as psum_pool,
        tc.tile_pool(name="outp", bufs=6) as out_pool,
    ):
        in_sb = cpool.tile([128, BL * IMG_STRIDE], F32)
        in_bf = cpool.tile([128, BL * IMG_STRIDE], BF16)
        w_raw = cpool.tile([128, 2 * W_FREE], F32)
        w_l = cpool.tile([128, N_TAPS * COUT], BF16)  # [ci, t*256 + h*128 + co]
        ident = cpool.tile([128, 128], F32)

        make_identity(nc, ident)

        # Warm-up: plain fp32 matmuls (4 cyc/row) on the identity keep the
        # PE busy through the weights-DMA wait so the HAM clock gate is
        # released before the conv stream starts.
        for _ in range(N_WARMUP):
            wps = psum_pool.tile([128, 512], F32, tag="ps")
            nc.tensor.matmul(wps[:, :128], ident, ident, start=True, stop=True)

        # DMA issue order = transfer completion order (queues drain FIFO):
        # h0 weights, image-0 staircase, h1 weights, remaining images.
        # (Issuing the weight DMAs on the Activation HW-DGE ring instead
        # was tried and regresses badly: conv cadence 209->251ns.)
        nc.sync.dma_start(
            out=w_raw[:, :W_FREE],
            in_=w_r.rearrange("(h p) c -> h p c", h=2)[0],
        )
        for c0, c1 in IMG0_PIECES:
            nc.sync.dma_start(out=in_sb[:, c0:c1], in_=x_r[0][:, c0:c1])
        nc.sync.dma_start(
            out=w_raw[:, W_FREE:],
            in_=w_r.rearrange("(h p) c -> h p c", h=2)[1],
        )
        for b in range(1, BL):
            nc.sync.dma_start(
                out=in_sb[:, b * IMG_STRIDE : (b + 1) * IMG_STRIDE],
                in_=x_r[b],
            )

        # fp32 -> bf16 input casts on the (otherwise idle) scalar engine,
        # piece-wise for image 0 so the conv stream can start early, and in
        # halves for the rest so each image's first row-blocks unblock after
        # ~2.1us of cast instead of 4.3us (removes a ~1us stall at image 1).
        for c0, c1 in IMG0_PIECES:
            nc.scalar.copy(in_bf[:, c0:c1], in_sb[:, c0:c1])
        for b in range(1, BL):
            for half in range(2):
                lo = b * IMG_STRIDE + half * (IMG_STRIDE // 2)
                hi = lo + IMG_STRIDE // 2
                nc.scalar.copy(in_bf[:, lo:hi], in_sb[:, lo:hi])

        # Transpose weights: w_raw half h viewed as [co, (ci t)] -> per tap
        # [co, ci] (ci at stride 9) -> PE transpose -> bf16 [ci, co].
        def transpose_half(h):
            w_v = w_raw[:, h * W_FREE : (h + 1) * W_FREE].rearrange(
                "p (ci t) -> p t ci", t=N_TAPS
            )
            for t in range(N_TAPS):
                ps = psum_pool.tile([128, 512], F32, tag="ps")
                nc.tensor.transpose(ps[:, :128], w_v[:, t, :], ident)
                nc.vector.tensor_copy(
                    w_l[:, t * COUT + h * 128 : t * COUT + h * 128 + 128],
                    ps[:, :128],
                )

        def conv_half(h):
            for b in range(BL):
                img_v = in_bf[
                    :, b * IMG_STRIDE : (b + 1) * IMG_STRIDE
                ].rearrange("p (r x) -> p r x", x=W)  # [128, 64, 64]
                for y0 in range(0, OH, ROWS_PER_CHUNK):
                    nrows = min(ROWS_PER_CHUNK, OH - y0)
                    size = nrows * OW
                    ps = psum_pool.tile([128, 512], F32, tag="ps")
                    ps_v = ps[:, :size].rearrange("p (r x) -> p r x", x=OW)
                    for t in range(N_TAPS):
                        kh, kw = divmod(t, KW)
                        lhsT = w_l[:, t * COUT + h * 128 : t * COUT + h * 128 + 128]
                        rhs = img_v[:, y0 + kh : y0 + kh + nrows, kw : kw + OW]
                        nc.tensor.matmul(
                            ps_v,
                            lhsT,
                            rhs,
                            start=(t == 0),
                            stop=(t == N_TAPS - 1),
                        )
                    ot = out_pool.tile([128, ROWS_PER_CHUNK * OW], F32)
                    nc.vector.tensor_copy(ot[:, :size], ps[:, :size])
                    nc.sync.dma_start(
                        out=out_d[b, h * 128 : (h + 1) * 128, y0 : y0 + nrows, :],
                        in_=ot[:, :size].rearrange("p (r x) -> p r x", x=OW),
                    )

        transpose_half(0)
        conv_half(0)
        transpose_half(1)
        conv_half(1)


def _elide_mm_ticks(nc):
    """Remove the per-MM tile-tick semaphore increments inside 9-tap matmul
    accumulation groups and renumber every wait threshold on that sem.

    Tile gives every instruction a completion increment on its engine's
    tick semaphore, but consumers only ever wait on the cumulative count
    reached at a group-final MM. Walrus requires UpdateValue == 1, so we
    drop the intermediate increments and rewrite all waits into the new
    (kept increments only) numbering. A wait that referenced an elided
    count is bumped to the next kept increment (the group-final MM), which
    completes at-or-after the original trigger point.
    """
    insts_all = [
        i for fn in nc.m.functions for blk in fn.blocks for i in blk.instructions
    ]

    pe_sem_ids = set()
    for i in insts_all:
        if isinstance(i, mybir.InstMatmult) and i.sync_info is not None:
            for u in i.sync_info.on_update:
                if u.update_mode == "sem-inc":
                    pe_sem_ids.add(u.id)
    assert len(pe_sem_ids) == 1, pe_sem_ids
    pe_sem = next(iter(pe_sem_ids))

    def pe_incs(i):
        if i.sync_info is None:
            return []
        return [
            u
            for u in i.sync_info.on_update
            if u.id == pe_sem and u.update_mode == "sem-inc"
        ]

    incs = [i for i in insts_all if pe_incs(i)]
    for i in incs:
        ups = pe_incs(i)
        assert len(ups) == 1 and ups[0].update_value == 1, (i.name, ups)

    waited = set()
    for i in insts_all:
        si = i.sync_info
        if si is None:
            continue
        for w in si.on_wait:
            if w.id == pe_sem:
                assert w.wait_mode == "sem-ge-imm" and w.wait_reg is None, w
                waited.add(w.wait_value)

    kept = []
    for old_cum, i in enumerate(incs, start=1):
        elide = (
            isinstance(i, mybir.InstMatmult)
            and not i.is_transpose
            and not i.stop_tensor_calc
            and old_cum not in waited
        )
        kept.append(not elide)
    newc = []
    c = 0
    for k in kept:
        c += k
        newc.append(c)

    n_elided = 0
    for i, k in zip(incs, kept):
        if not k:
            si = i.sync_info
            i.sync_info = bass_rust.SyncInfo(
                on_wait=list(si.on_wait),
                on_update=[
                    u
                    for u in si.on_update
                    if not (u.id == pe_sem and u.update_mode == "sem-inc")
                ],
            )
            n_elided += 1

    for i in insts_all:
        si = i.sync_info
        if si is None:
            continue
        for w in si.on_wait:
            if w.id == pe_sem:
                v = w.wait_value
                assert 1 <= v <= len(incs), (i.name, v)
                w.wait_value = newc[v - 1] + (0 if kept[v - 1] else 1)
    return n_elided


def build_module(elide_ticks=ELIDE):
    nc = bacc.Bacc(
        "TRN2", target_bir_lowering=False, debug=False, num_devices=N_CORES
    )
    x_d = nc.dram_tensor(
        "input_image", [BL, CIN, H, W], F32, kind="ExternalInput"
    ).ap()
    w_d = nc.dram_tensor("weights", [COUT, CIN, KH, KW], F32, kind="ExternalInput").ap()
    out_d = nc.dram_tensor("out", [BL, COUT, OH, OW], F32, kind="ExternalOutput").ap()
    with tile.TileContext(nc) as tc:
        _conv_body(nc, tc, out_d, x_d, w_d)
    if elide_ticks:
        try:
            _elide_mm_ticks(nc)
        except Exception:
            # the pass mutates sync_info in place; on any unexpected BIR
            # shape discard this module and rebuild without the elision
            return build_module(elide_ticks=False)
    nc.compile()
    return nc


_NC_CACHE = {}


def _get_module():
    if "nc" not in _NC_CACHE:
        _NC_CACHE["nc"] = build_module()
    return _NC_CACHE["nc"]


def kernel(input_image: np.ndarray, weights: np.ndarray) -> np.ndarray:
    input_image = np.ascontiguousarray(input_image, dtype=np.float32)
    weights = np.ascontiguousarray(weights, dtype=np.float32)
    nc = _get_module()
    in_maps = [
        {
            "input_image": input_image[i * BL : (i + 1) * BL],
            "weights": weights,
        }
        for i in range(N_CORES)
    ]
    res = run_bass_kernel_spmd(nc, in_maps, list(range(N_CORES))).results
    return np.concatenate([r["out"] for r in res], axis=0)
